# revision 18
# baseline (speedup 1.0000x reference)
"""BitConvBlock kernel for 8x Trainium2 NeuronCores (SPMD, batch-sharded).

Reference computation (per sample):
  Wq = ternary-quantized W (BitNet b1.58: s = mean|W|, T = clip(round(W/(s+eps)),-1,1), Wq = s*T)
  y  = conv1d(x, Wq, pad=3)                      [B=16, Cout=512, L=8192]
  yn = GroupNorm(1 group, per-channel affine)(y)
  out= yn + sin(alpha*yn + phase)^2 / (alpha+eps)

Strategy (v2):
  - Batch-parallel: 16 samples / 8 cores = 2 samples per core. GroupNorm is
    per-sample, so no collectives.
  - Conv as matmul in bf16 single-pass: x cast to bf16 host-side; ternary
    weights are exact in bf16; scale s folded into the GN epsilon. End-to-end
    rel_max ~3e-3 (gate 2e-2). bf16 runs the PE at 1 col/cycle, the same rate
    as fp32r/fp8-DoubleRow-hi-lo, but halves x DMA/SBUF.
  - y is kept in SBUF as bf16 (128 KB/partition for both samples): no DRAM
    spill round-trip. GN stats accumulate in fp32 from PSUM during phase A.
  - Phase B (GN affine + snake), 7 passes/element spread over 3 engines
    (custom-DVE fusions fail to compile on this walrus, so standard ops):
      VEC u   = a2*y + b2                  (turns; a2 = alpha*gamma/(2pi*std))
      ACT z   = Identity(a2*y + (b2+MAGIC))  (magic round-to-nearest)
      VEC red = (z - MAGIC) - u            (reduced turns, sign-flipped)
      ACT sg  = Sin(2pi*red)
      ACT sq2 = Square(sqi*sg)             (sqi^2 = 1/(alpha+eps))
      VEC yn  = y*A + B
      POOL out= yn + sq2                   (gpsimd; VEC if BITCONV_POOLADD=0)
  - Emission interleaves sample 0's phase B between sample 1's conv windows
    so VEC/ACT phase-B work hides under the PE; only the last sample's
    phase B (~90 us) is exposed.
"""
import os
import numpy as np
import ml_dtypes
from contextlib import ExitStack

# ---------------------------------------------------------------- constants
B, CIN, COUT, K, L = 16, 512, 512, 7, 8192
PAD = 3
EPS_Q, EPS_GN, EPS_A = 1e-5, 1e-5, 1e-9
NCORE = 8
BPC = B // NCORE          # samples per core
NCT = COUT // 128         # 4 co tiles
NCI = CIN // 128          # 4 ci tiles
LW = 512                  # conv l-window (one fp32 PSUM bank)
NLW = L // LW             # 16 windows
LP = L + 2 * PAD          # padded length 8198
BW = 512                  # phase-B tile width
NBW = L // BW             # 8 phase-B tiles per (sample, co_t)
NBCH = NCT * NBW          # 32 phase-B chunks per sample
NELEM = COUT * L          # GN reduction size per sample
PI = 3.141592653589793
TWO_PI = 6.283185307179586
INV_2PI = 1.0 / TWO_PI
MAGIC = 12582912.0        # 1.5 * 2**23: float32 round-to-nearest-even trick
POOL_ADD = bool(int(os.environ.get("BITCONV_POOLADD", "1")))

_last_results = {}


def _ternary(W: np.ndarray):
    """Bit-exact replica of the reference's _ternary_quant (value part)."""
    s = None
    try:
        import jax
        import jax.numpy as jnp

        cpus = jax.devices("cpu")
        with jax.default_device(cpus[0]):
            s = float(jnp.mean(jnp.abs(jnp.asarray(W))))
    except Exception:
        s = float(np.mean(np.abs(W), dtype=np.float32))
    s32 = np.float32(s)
    q = (W / (s32 + np.float32(EPS_Q))).astype(np.float32)
    T = np.clip(np.rint(q), -1.0, 1.0).astype(np.float32)
    return T, float(s32)


def _build_and_run(in_maps, eps_eff: float, trace: bool):
    import concourse.bass as bass
    import concourse.tile as tile
    import concourse.mybir as mybir

    # walrus here accepts only one sync-wait per instruction; split waits.
    import bass_rust
    from concourse.vector_clock import ScopedClock, VectorClock

    _orig_commit = getattr(tile.TileContext, "_bitconv_orig_commit", None)
    if _orig_commit is None:
        _orig_commit = tile.TileContext._commit_and_lower
        tile.TileContext._bitconv_orig_commit = _orig_commit
    _skip = (tile.BassTileRelease, tile.BassTileBranchHintPlaceholder,
             tile.BassTileCriticalSection)

    def _commit_split(self, inst, original_block, old_bb_map, bb_to_exit_bb):
        si = getattr(inst, "sync_info", None)
        if (si is not None and len(si.on_wait) > 1
                and not isinstance(inst, _skip)
                and not bass.is_branch_inst(inst)
                and inst.engine != mybir.EngineType.Unassigned):
            waits = list(si.on_wait)
            plain = [w for w in waits
                     if w.sync_type == "semaphore" and w.wait_reg is None]
            rest = [w for w in waits
                    if not (w.sync_type == "semaphore" and w.wait_reg is None)]
            if len(rest) <= 1 and plain:
                keep = rest if rest else [plain.pop()]
                for w in plain:
                    ev = mybir.InstEventSemaphore(
                        name=self.nc.get_next_instruction_name(), ins=[], outs=[])
                    ev.engine = inst.engine
                    ev.sync_info = bass_rust.SyncInfo(on_wait=[w], on_update=[])
                    self._commit_instruction(ev, lazy_reg_writes=False)
                inst.sync_info = bass_rust.SyncInfo(
                    on_wait=keep, on_update=list(si.on_update))
        return _orig_commit(self, inst, original_block, old_bb_map, bb_to_exit_bb)

    def _drain_split(self, tick_clock, wait_clock):
        g = tick_clock.global_clock
        n = len(g)
        for p in range(n):
            t = g[p]
            if t == 0:
                continue
            vec = [0] * n
            vec[p] = t
            d = self.nc.sync.drain()
            wait_clock.add_sem_waits(d.ins, ScopedClock({None: VectorClock(vec)}))
        self.nc.sync.drain()
        self.nc.all_engine_barrier()
        assert self.sems is not None
        popped = self.nc._tile_sem_poison_stack.pop()
        assert popped is self._sem_poison
        self.nc.clear_and_free_semaphores(list(self.sems.allocated().values()))
        self.nc.all_engine_barrier()

    tile.TileContext._commit_and_lower = _commit_split
    tile.TileContext._drain_and_barrier = _drain_split

    from concourse.bass_utils import run_bass_kernel_spmd

    f32 = mybir.dt.float32
    bf16 = mybir.dt.bfloat16
    AF = mybir.ActivationFunctionType
    ALU = mybir.AluOpType
    AX = mybir.AxisListType

    nc = bass.Bass("TRN2", target_bir_lowering=False, debug=False)

    # DRAM I/O. x layout is partition-major so a whole conv window (all 4
    # ci tiles) moves in ONE dma: [BPC, 128, NCI, LP].
    x_in = nc.dram_tensor("xb", [BPC, 128, NCI, LP], bf16, kind="ExternalInput").ap()
    w_in = nc.dram_tensor("Wt", [128, K * NCI * NCT * 128], bf16, kind="ExternalInput").ap()
    cc_in = nc.dram_tensor("cc", [128, 24], f32, kind="ExternalInput").ap()
    id_in = nc.dram_tensor("idm", [128, 128], bf16, kind="ExternalInput").ap()
    out_ap = nc.dram_tensor("out", [BPC, NCT, 128, L], f32, kind="ExternalOutput").ap()

    wchunk = K * NCI * 128

    def widx(k, ci):
        return (k * NCI + ci) * 128

    with tile.TileContext(nc) as tc:
        with ExitStack() as ctx:
            wpool = ctx.enter_context(tc.tile_pool(name="w", bufs=1))
            cpool = ctx.enter_context(tc.tile_pool(name="consts", bufs=1))
            xpool = ctx.enter_context(tc.tile_pool(name="x", bufs=3))
            cps = ctx.enter_context(tc.tile_pool(name="cps", bufs=7, space="PSUM"))
            sps = ctx.enter_context(tc.tile_pool(name="sps", bufs=1, space="PSUM"))
            ypool = ctx.enter_context(tc.tile_pool(name="ysb", bufs=2))
            qpool = ctx.enter_context(tc.tile_pool(name="sqd", bufs=2))
            stpool = ctx.enter_context(tc.tile_pool(name="st", bufs=2))
            smpool = ctx.enter_context(tc.tile_pool(name="sm", bufs=2))
            bpool = ctx.enter_context(tc.tile_pool(name="bp", bufs=3))

            _x00_holder = []
            W_sbs = []
            _wq = [nc.sync, nc.scalar, nc.gpsimd, nc.scalar]
            _x00_holder.append(
                xpool.tile([128, NCI, LW + 2 * PAD], bf16, name="xw0_0", tag="xw"))
            nc.sync.dma_start(_x00_holder[0][:], x_in[0, :, :, 0:LW + 2 * PAD])
            WSPLIT = 4
            wsub = wchunk // WSPLIT       # 7 (k,ci) pairs of 128 cols each
            qi = 0
            for ct in range(NCT):
                subs = []
                for sl in range(WSPLIT):
                    wt = wpool.tile([128, wsub], bf16,
                                    name=f"wsb{ct}_{sl}", tag=f"w{ct}_{sl}")
                    a = ct * wchunk + sl * wsub
                    _wq[qi % len(_wq)].dma_start(wt[:], w_in[:, a:a + wsub])
                    qi += 1
                    subs.append(wt)
                W_sbs.append(subs)
            cc_sb = cpool.tile([128, 24], f32)
            nc.sync.dma_start(cc_sb[:], cc_in[:])
            id_sb = cpool.tile([128, 128], bf16)
            nc.sync.dma_start(id_sb[:], id_in[:])
            gnw_c = cc_sb[:, 0:NCT]
            gnb_c = cc_sb[:, NCT:2 * NCT]
            alp_c = cc_sb[:, 2 * NCT:3 * NCT]
            phs_c = cc_sb[:, 3 * NCT:4 * NCT]
            sqi_c = cc_sb[:, 4 * NCT:5 * NCT]
            sqi2_c = cc_sb[:, 5 * NCT:6 * NCT]
            ones_sb = cpool.tile([128, 128], f32)
            nc.vector.memset(ones_sb[:], 1.0)

            # persistent state created per sample
            ys = [[None] * NCT for _ in range(BPC)]       # y in SBUF, bf16
            affs = [None] * BPC                           # (Av, Bv, a2v, b2v)

            def fetch_x(smp, lw):
                l0 = lw * LW
                xt = xpool.tile([128, NCI, LW + 2 * PAD], bf16,
                                name=f"xw{smp}_{lw}", tag="xw")
                q = nc.sync if lw % 2 == 0 else nc.scalar
                q.dma_start(xt[:], x_in[smp, :, :, l0:l0 + LW + 2 * PAD])
                return xt

            def emit_phase_a_window(smp, lw, xt=None):
                l0 = lw * LW
                if xt is None:
                    xt = fetch_x(smp, lw)
                st_sb = st_tiles[smp]
                for ct in range(NCT):
                    ps = cps.tile([128, LW], f32, tag="cpsum")
                    for ci in range(NCI):
                        for k in range(K):
                            flat = k * NCI + ci
                            w_ap = W_sbs[ct][flat // 7][:, (flat % 7) * 128:
                                                        (flat % 7) * 128 + 128]
                            first = ci == 0 and k == 0
                            last = ci == NCI - 1 and k == K - 1
                            nc.tensor.matmul(ps[:], w_ap, xt[:, ci, k:k + LW],
                                             start=first, stop=last)
                    idx = ct * NLW + lw
                    nc.vector.tensor_scalar(
                        ys[smp][ct][:, l0:l0 + LW], ps[:], 1.0, 0.0,
                        ALU.mult, ALU.add,
                        accum_out=st_sb[:, idx:idx + 1])
                    sqd = qpool.tile([128, LW], bf16, tag="sqd")
                    nc.scalar.activation(
                        sqd[:], ps[:], AF.Square,
                        accum_out=st_sb[:, NCT * NLW + idx:NCT * NLW + idx + 1])

            def emit_stats(smp):
                st_sb = st_tiles[smp]
                red = smpool.tile([128, 2], f32, tag="red")
                nc.vector.reduce_sum(red[:, 0:1], st_sb[:, 0:NCT * NLW], axis=AX.X)
                nc.vector.reduce_sum(red[:, 1:2], st_sb[:, NCT * NLW:2 * NCT * NLW], axis=AX.X)
                stps = sps.tile([128, 2], f32, tag="stps")
                nc.tensor.matmul(stps[:], ones_sb[:], red[:, 0:2], start=True, stop=True)
                mv = smpool.tile([128, 2], f32, tag="mv")
                nc.vector.tensor_scalar_mul(mv[:], stps[:], 1.0 / NELEM)
                musq = smpool.tile([128, 1], f32, tag="musq")
                nc.vector.tensor_mul(musq[:], mv[:, 0:1], mv[:, 0:1])
                var = smpool.tile([128, 1], f32, tag="var")
                nc.vector.tensor_sub(var[:], mv[:, 1:2], musq[:])
                nc.vector.tensor_scalar_add(var[:], var[:], float(eps_eff))
                std = smpool.tile([128, 1], f32, tag="std")
                nc.scalar.activation(std[:], var[:], AF.Sqrt)
                rv = smpool.tile([128, 1], f32, tag="rv")
                nc.vector.reciprocal(rv[:], std[:])
                Av = smpool.tile([128, NCT], f32, tag="Av")
                nc.vector.tensor_scalar_mul(Av[:], gnw_c, rv[:])
                negmu = smpool.tile([128, 1], f32, tag="negmu")
                nc.vector.tensor_scalar_mul(negmu[:], mv[:, 0:1], -1.0)
                Bv = smpool.tile([128, NCT], f32, tag="Bv")
                nc.vector.tensor_scalar_mul(Bv[:], Av[:], negmu[:])
                nc.vector.tensor_add(Bv[:], Bv[:], gnb_c)
                # u = (alpha*yn + phase)/2pi = a2*y + b2 (turns)
                a2v = smpool.tile([128, NCT], f32, tag="a2v")
                nc.vector.tensor_mul(a2v[:], alp_c, Av[:])
                b2v = smpool.tile([128, NCT], f32, tag="b2v")
                nc.vector.tensor_mul(b2v[:], alp_c, Bv[:])
                nc.vector.tensor_add(b2v[:], b2v[:], phs_c)
                b2vm = smpool.tile([128, NCT], f32, tag="b2vm")
                nc.vector.tensor_scalar_add(b2vm[:], b2v[:], MAGIC)
                dga = dgy = None
                if smp == BPC - 1:
                    dga = smpool.tile([128, NCT, 128], bf16, name=f"dga{smp}", tag="dga")
                    dgy = smpool.tile([128, NCT, 128], bf16, name=f"dgy{smp}", tag="dgy")
                    for ct in range(NCT):
                        nc.vector.tensor_scalar(dga[:, ct, :], id_sb[:],
                                                a2v[:, ct:ct + 1], 0.0,
                                                ALU.mult, ALU.add)
                        nc.vector.tensor_scalar(dgy[:, ct, :], id_sb[:],
                                                Av[:, ct:ct + 1], 0.0,
                                                ALU.mult, ALU.add)
                affs[smp] = (Av, Bv, a2v, b2v, b2vm, dga, dgy)

            _tail_q = []

            def _flush_tail():
                smp, ct, g, ynps, Bv, sq2 = _tail_q.pop(0)
                outt = bpool.tile([128, BW], f32, tag="outt")
                nc.vector.scalar_tensor_tensor(outt[:], ynps[:],
                                               Bv[:, ct:ct + 1], sq2[:],
                                               ALU.add, ALU.add)
                nc.sync.dma_start(out_ap[smp, ct, :, g * BW:(g + 1) * BW],
                                  outt[:])

            def emit_phase_b_chunk(smp, j, use_pe=False):
                ct, g = j // NBW, j % NBW
                Av, Bv, a2v, b2v, b2vm, dga, dgy = affs[smp]
                if use_pe:
                    # tail variant: PE computes the two per-channel affines
                    # as diag matmuls into PSUM (PE is otherwise idle here);
                    # biases fold into the AP-scalar slots downstream.
                    ysb = ys[smp][ct][:, g * BW:(g + 1) * BW]
                    ups = cps.tile([128, BW], f32, tag="cpsum")
                    nc.tensor.matmul(ups[:], dga[:, ct, :], ysb,
                                     start=True, stop=True)
                    ynps = cps.tile([128, BW], f32, tag="cpsum")
                    nc.tensor.matmul(ynps[:], dgy[:, ct, :], ysb,
                                     start=True, stop=True)
                    z = bpool.tile([128, BW], f32, tag="z")
                    if j % 3 == 0:
                        nc.vector.tensor_scalar(z[:], ups[:], 1.0,
                                                b2vm[:, ct:ct + 1],
                                                ALU.mult, ALU.add)
                    else:
                        nc.scalar.activation(z[:], ups[:], AF.Identity,
                                             bias=b2vm[:, ct:ct + 1])
                    nc.vector.scalar_tensor_tensor(
                        z[:], z[:], b2vm[:, ct:ct + 1], ups[:],
                        ALU.subtract, ALU.subtract)
                    sg = bpool.tile([128, BW], bf16, tag="sg")
                    nc.scalar.activation(sg[:], z[:], AF.Sin, scale=TWO_PI)
                    sq2 = bpool.tile([128, BW], bf16, tag="sq2")
                    if j % 8 == 1:
                        nc.vector.scalar_tensor_tensor(
                            sq2[:], sg[:], sqi2_c[:, ct:ct + 1], sg[:],
                            ALU.mult, ALU.mult)
                    else:
                        nc.scalar.activation(sq2[:], sg[:], AF.Square,
                                             scale=sqi_c[:, ct:ct + 1])
                    _tail_q.append((smp, ct, g, ynps, Bv, sq2))
                    if len(_tail_q) > 1:
                        _flush_tail()
                    return
                ysb = ys[smp][ct][:, g * BW:(g + 1) * BW]
                u = bpool.tile([128, BW], f32, tag="u")
                nc.vector.tensor_scalar(u[:], ysb, a2v[:, ct:ct + 1],
                                        b2v[:, ct:ct + 1], ALU.mult, ALU.add)
                z = bpool.tile([128, BW], f32, tag="z")
                nc.scalar.activation(z[:], ysb, AF.Identity,
                                     bias=b2vm[:, ct:ct + 1],
                                     scale=a2v[:, ct:ct + 1])
                # in-place: z <- (z - MAGIC) - u  (reduced turns, sign flip ok)
                nc.vector.scalar_tensor_tensor(
                    z[:], z[:], MAGIC, u[:], ALU.subtract, ALU.subtract)
                sg = bpool.tile([128, BW], bf16, tag="sg")
                nc.scalar.activation(sg[:], z[:], AF.Sin, scale=TWO_PI)
                sq2 = bpool.tile([128, BW], bf16, tag="sq2")
                nc.scalar.activation(sq2[:], sg[:], AF.Square,
                                     scale=sqi_c[:, ct:ct + 1])
                yn = bpool.tile([128, BW], f32, tag="yn")
                nc.vector.tensor_scalar(yn[:], ysb, Av[:, ct:ct + 1],
                                        Bv[:, ct:ct + 1], ALU.mult, ALU.add)
                outt = bpool.tile([128, BW], f32, tag="outt")
                eng = nc.gpsimd if POOL_ADD else nc.vector
                eng.tensor_add(outt[:], yn[:], sq2[:])
                nc.sync.dma_start(out_ap[smp, ct, :, g * BW:(g + 1) * BW], outt[:])

            st_tiles = []
            for smp in range(BPC):
                st_tiles.append(stpool.tile([128, 2 * NCT * NLW], f32, name=f"st{smp}", tag="st"))
                for ct in range(NCT):
                    ys[smp][ct] = ypool.tile([128, L], bf16, name=f"y{smp}_{ct}", tag=f"y{ct}")

            # sample 0 conv
            for lw in range(NLW):
                emit_phase_a_window(0, lw, xt=_x00_holder[0] if lw == 0 else None)
            emit_stats(0)
            # sample 1 conv with sample 0's phase B interleaved (2 chunks
            # per window: 32 chunks over 16 windows)
            cpw = NBCH // NLW   # phase-B chunks interleaved per window
            for lw in range(NLW):
                emit_phase_a_window(1, lw)
                for q in range(cpw):
                    emit_phase_b_chunk(0, cpw * lw + q)
            emit_stats(1)
            for j in range(NBCH):
                emit_phase_b_chunk(1, j, use_pe=True)
            while _tail_q:
                _flush_tail()

    if trace:
        _install_profile_shim()
    res = run_bass_kernel_spmd(nc, in_maps, list(range(NCORE)), trace=trace)
    return res


def _install_profile_shim():
    """Register antenv.axon_hooks so trace=True captures NTFF profiles via the
    axon .so (profiling only; never needed for plain execution)."""
    import sys, types, importlib.util

    if "antenv.axon_hooks" in sys.modules:
        return
    try:
        holder = {"hook": None}
        mod = types.ModuleType("antenv.axon_hooks")
        mod.set_axon_ntff_profile_hook = lambda h: holder.__setitem__("hook", h)
        mod.get_axon_ntff_profile_hook = lambda: holder["hook"]
        import antenv

        spec = importlib.util.spec_from_file_location(
            "trn_boot_shim", "/root/.axon_site/trn_agent_boot/trn_boot.py")
        boot = importlib.util.module_from_spec(spec)
        spec.loader.exec_module(boot)
        hook = boot._ntff_profile_via_ctypes("/opt/axon/libaxon_pjrt.so")
        if hook is None:
            return
        mod.set_axon_ntff_profile_hook(hook)
        sys.modules["antenv.axon_hooks"] = mod
        antenv.axon_hooks = mod
    except Exception:
        pass


def kernel(x, W, gn_w, gn_b, alpha, phase):
    x = np.asarray(x, dtype=np.float32)
    W = np.asarray(W, dtype=np.float32)
    gn_w = np.asarray(gn_w, dtype=np.float32)
    gn_b = np.asarray(gn_b, dtype=np.float32)
    alpha = np.asarray(alpha, dtype=np.float32)
    phase = np.asarray(phase, dtype=np.float32)

    trace = bool(int(os.environ.get("BITCONV_TRACE", "0")))

    T, s = _ternary(W)   # T in {-1,0,1}, conv scale s folded into GN eps
    eps_eff = float(EPS_GN / (np.float64(s) ** 2))

    # weight layout: Wt[ci_in_tile, (co_t, k, ci_t, co)] = T[co, ci, k]
    Tr = T.reshape(NCT, 128, NCI, 128, K)          # [co_t, co, ci_t, ci, k]
    Wt = np.ascontiguousarray(Tr.transpose(3, 0, 4, 2, 1)).reshape(128, -1)
    Wt = Wt.astype(ml_dtypes.bfloat16)

    # padded activations, bf16, partition-major: [B, 128, NCI, LP]
    xp = np.zeros((B, CIN, LP), dtype=ml_dtypes.bfloat16)
    xp[:, :, PAD:PAD + L] = x.astype(ml_dtypes.bfloat16)
    xp = np.ascontiguousarray(
        xp.reshape(B, NCI, 128, LP).transpose(0, 2, 1, 3))

    # per-channel constants [128, col]
    def tilec(v):
        return np.ascontiguousarray(v.reshape(NCT, 128).T)  # [128, NCT]

    sqinv = np.sqrt(1.0 / (alpha.astype(np.float64) + EPS_A)).astype(np.float32)
    cc = np.zeros((128, 24), dtype=np.float32)
    cc[:, 0:NCT] = tilec(gn_w)
    cc[:, NCT:2 * NCT] = tilec(gn_b)
    cc[:, 2 * NCT:3 * NCT] = tilec((alpha.astype(np.float64) / (2 * np.pi)).astype(np.float32))
    cc[:, 3 * NCT:4 * NCT] = tilec((phase.astype(np.float64) / (2 * np.pi)).astype(np.float32))
    cc[:, 4 * NCT:5 * NCT] = tilec(sqinv)
    cc[:, 5 * NCT:6 * NCT] = tilec((1.0 / (alpha.astype(np.float64) + EPS_A)).astype(np.float32))

    idm = np.eye(128, dtype=ml_dtypes.bfloat16)
    in_maps = []
    for c in range(NCORE):
        in_maps.append({
            "xb": np.ascontiguousarray(xp[c * BPC:(c + 1) * BPC]),
            "Wt": Wt,
            "cc": cc,
            "idm": idm,
        })

    res = _build_and_run(in_maps, eps_eff, trace)
    _last_results["exec_time_ns"] = res.exec_time_ns
    _last_results["mean_exec_time_ns"] = res.mean_exec_time_ns

    out = np.empty((B, COUT, L), dtype=np.float32)
    for c in range(NCORE):
        o = res.results[c]["out"]          # [BPC, NCT, 128, L]
        out[c * BPC:(c + 1) * BPC] = o.reshape(BPC, COUT, L)
    return out
